# revision 33
# baseline (speedup 1.0000x reference)
"""Trainium2 Bass kernel for nn_Decoder (single-query MHA + pointer head).

Contract: kernel(**inputs) takes the FULL unsharded numpy inputs (as produced
by the problem's setup_inputs) and returns the full output (vertexes, probs),
matching the reference up to fp32 rounding.

v9 strategy (pure data parallelism over batch, 8 NeuronCores, 32 batch each):
  - Host-side compaction: mask kills ~50% of the N=1024 positions (score
    -1e15 -> attn weight 0; pointer logit -1e15 -> never argmax), so only
    the unmasked K/V/K_lg columns are shipped, padded to N_k (multiple of
    64, 576 for the seed-0 inputs).  Pad columns are zero + a -1e15 bias.
    Original vertex ids are recovered on-device from an index table via
    (logit == rowmax) * idx -> reduce-max.
  - All scores and pointer logits run on the TensorEngine as PSUM
    accumulation chains with zero-padded per-batch stationaries.  Scores
    use [128, 32] stationaries on tile_position col-bands (pitch-24 flat
    windows put batch 4g+jj's q at band 32g, local col 8jj+h -> psum
    partition 8j+h); emission is jj-outer so the four bands run
    concurrently in the PE array.  The pad bias is accumulated by one
    bf16 selector matmul per bank, so each block's scores drain with one
    DVE reduce (negmax) + one ACT exp.  A burst of junk bf16 matmuls
    warms the PE's HAM clock gate before the first score chain.
  - V contraction: 12 d's as DVE stt+accum, 4 d's as DVE tensor_tensor
    product + ACT Copy+accum_out, balancing the two engines.
  - Pointer head runs per 16-batch block ([16, N_k] PSUM reusing the
    score banks), so block 0's tanh/softmax/argmax tail hides under the
    DMA stream and only block 1's short tail trails the last K_lg tile
    (a 4-batch group).  exp bias comes from negating max8's first lane.
  - One HWDGE DMA ring (sync) carries all bulk loads in need order
    (~420 GB/s observed); V ships as four quarter tiles so the first stt
    can start earlier; u round-trips are one-hop SBUF->SBUF DMAs on the
    scalar HWDGE ring.
"""

import numpy as np

B, N, D, H, HD = 256, 1024, 128, 8, 16
NCORES = 8
BPC = B // NCORES          # 32 batches per core
BLK_B = 16                 # batches per score-block (16 b x 8 h = 128 rows)
GRP = 8                    # batches per K^T DMA group tile
GRPL = 4                   # batches per K_lg DMA group tile
KPAD = 512                 # 386 -> 512 (4 chunks of 128) for Q projection
NEG = -30.0                # pad bias: exp(-30+s-max) underflows fp16 to 0
                           # exactly in the attn softmax, and tanh(-30) = -1
                           # exactly in f32, so pads sit at pointer logit -10
                           # (strictly below every real 10*tanh) and add only
                           # ~exp(-10-max) to the pointer softmax sum.
RSQ_D = float(1.0 / np.sqrt(128.0))
NCONST = 1034              # ident|woT|bq|bo|hcT|wqT|hmask|sel16
NDVE = 16                  # V-contraction d's on DVE stt (gpsimd tensor ops
                           # concurrent with DVE wedge the device; ACT-assist
                           # is slower than stt: ~1040ns vs 755ns per d)

_PROG_CACHE = {}
_SHARED_CACHE = {}


def _build_program(NK):
    import concourse.bass as bass
    import concourse.bacc as bacc
    import concourse.mybir as mybir
    from concourse.tile import TileContext

    f32 = mybir.dt.float32
    f16 = mybir.dt.float16
    bf16 = mybir.dt.bfloat16
    Alu = mybir.AluOpType
    Act = mybir.ActivationFunctionType
    Ax = mybir.AxisListType

    # psum bank chunks of the N_k columns
    chunks = [(0, min(512, NK))]
    if NK > 512:
        chunks.append((512, NK))

    nc = bacc.Bacc(None, target_bir_lowering=False)

    consts = nc.declare_dram_parameter("consts", [128, NCONST], f32,
                                       isOutput=False)
    KtG = nc.declare_dram_parameter("KtG", [4, 128, GRP * NK], f16,
                                    isOutput=False)
    Vt = nc.declare_dram_parameter("Vt", [4, 128, (HD // 2) * NK], f16,
                                   isOutput=False)
    KlgG = nc.declare_dram_parameter("KlgG", [8, 128, GRPL * NK], f16,
                                     isOutput=False)
    m32b = nc.declare_dram_parameter("m32b", [BLK_B, 2 * NK], f32,
                                     isOutput=False)
    idxt = nc.declare_dram_parameter("idxt", [BLK_B, 2 * NK], f32,
                                     isOutput=False)
    vert_out = nc.declare_dram_parameter("verts", [BPC, 1], f32, isOutput=True)
    probs_out = nc.declare_dram_parameter("probs", [BPC, 1], f32, isOutput=True)

    with TileContext(nc) as tc:
        import contextlib

        with contextlib.ExitStack() as ctx:
            const_p = ctx.enter_context(tc.tile_pool(name="const", bufs=1))
            small_p = ctx.enter_context(tc.tile_pool(name="small", bufs=1))
            ktp = ctx.enter_context(tc.tile_pool(name="ktp", bufs=4))
            vtp = ctx.enter_context(tc.tile_pool(name="vtp", bufs=4))
            klgp = ctx.enter_context(tc.tile_pool(name="klgp", bufs=8))
            e2p = ctx.enter_context(tc.tile_pool(name="e2p", bufs=2))
            junk_p = ctx.enter_context(tc.tile_pool(name="junk", bufs=3))
            junk_a = ctx.enter_context(tc.tile_pool(name="junka", bufs=3))
            junk_g = ctx.enter_context(tc.tile_pool(name="junkg", bufs=3))
            upl_p = ctx.enter_context(tc.tile_pool(name="upl", bufs=2))
            tail_p = ctx.enter_context(tc.tile_pool(name="tail", bufs=2))
            psq = ctx.enter_context(
                tc.tile_pool(name="psq", bufs=2, space=bass.MemorySpace.PSUM))
            psS = ctx.enter_context(
                tc.tile_pool(name="psS", bufs=2, space=bass.MemorySpace.PSUM))

            # ====== DMA: one sync-ring stream in need order ======
            cblob = const_p.tile([128, NCONST], f32, name="cblob")
            nc.sync.dma_start(cblob[:], consts[:])
            m32b_t = small_p.tile([BLK_B, 2 * NK], f32)
            nc.sync.dma_start(m32b_t[:], m32b[:])
            kt_t = [ktp.tile([128, GRP * NK], f16, name="kt_t")
                    for _ in range(4)]
            vt_t = [vtp.tile([128, (HD // 2) * NK], f16, name="vt_t")
                    for _ in range(4)]
            klg_t = [klgp.tile([128, GRPL * NK], f16, name="klg_t")
                     for _ in range(8)]
            nc.sync.dma_start(kt_t[0][:], KtG[0])
            nc.sync.dma_start(kt_t[1][:], KtG[1])
            nc.sync.dma_start(vt_t[0][:], Vt[0])
            nc.sync.dma_start(vt_t[1][:], Vt[1])
            nc.sync.dma_start(kt_t[2][:], KtG[2])
            nc.sync.dma_start(kt_t[3][:], KtG[3])
            nc.sync.dma_start(vt_t[2][:], Vt[2])
            nc.sync.dma_start(vt_t[3][:], Vt[3])
            for g in range(8):
                nc.sync.dma_start(klg_t[g][:], KlgG[g])
            idxt_t = small_p.tile([BLK_B, 2 * NK], f32)
            nc.sync.dma_start(idxt_t[:], idxt[:])

            ident_t = cblob[:, 0:128]
            wo_t = cblob[:, 128:256]
            bq_t = cblob[:, 256:257]
            bo_t = cblob[:, 257:258]
            hc_t = cblob[:, 258:386].rearrange("p (c b) -> p c b", b=BPC)
            wq_t = cblob[:, 386:898].rearrange("p (c d) -> p c d", d=D)
            hmask_t = cblob[:, 898:906]

            # preload the ACT function tables off the critical path
            dummy = small_p.tile([1, 16], f32)
            nc.vector.memset(dummy[:], 0)
            nc.scalar.activation(dummy[:], dummy[:], Act.Exp)
            nc.scalar.activation(dummy[:], dummy[:], Act.Tanh)

            # bf16 casts for the pad-bias matmul operands
            sel16b = const_p.tile([BLK_B, 128], bf16)
            nc.vector.tensor_copy(sel16b[:], cblob[0:BLK_B, 906:1034])
            id16b = const_p.tile([BLK_B, BLK_B], bf16)
            nc.vector.tensor_copy(id16b[:], cblob[0:BLK_B, 0:BLK_B])
            m32b16 = small_p.tile([BLK_B, 2 * NK], bf16)
            nc.vector.tensor_copy(m32b16[:], m32b_t[:])

            # ====== Q projection -> qt_s = 0.25*(Q^T + bq)  [(h d), b] ======
            qp_ps = psq.tile([128, 512], f32, name="qp_ps")
            for kc in range(KPAD // 128):
                nc.tensor.matmul(
                    qp_ps[:, 0:BPC], wq_t[:, kc, :], hc_t[:, kc, :],
                    start=(kc == 0), stop=(kc == KPAD // 128 - 1))
            bq25 = const_p.tile([D, 1], f32)
            nc.vector.tensor_scalar_mul(bq25[:], bq_t, 0.25)
            bo_s = const_p.tile([D, 1], f32)
            nc.vector.tensor_scalar_mul(bo_s[:], bo_t, RSQ_D)
            qt_s = small_p.tile([D, BPC], f32)
            nc.vector.scalar_tensor_tensor(
                out=qt_s[:], in0=qp_ps[:, 0:BPC], scalar=0.25,
                in1=bq25[:, 0:1].broadcast_to([D, BPC]),
                op0=Alu.mult, op1=Alu.add)

            # ====== zero-padded stationaries ======
            # scores: block-local batch j = 4g+jj -> band g (tile_position
            # (0,32g)), window qflat[:, 128g+24jj : +32], nonzero flat col
            # 128g+32jj+h = local col 8jj+h -> psum partition 8j+h.
            qflat = [small_p.tile([128, 512], f16, name=f"qflat{b}")
                     for b in range(2)]
            # logits: batch j at flat col 16j inside a pitch-15 window
            # [15j, 15j+16) -> local col j -> psum partition j.
            u2flat = [small_p.tile([128, 256], f16, name=f"u2flat{b}")
                      for b in range(2)]
            for b in range(2):
                nc.gpsimd.memset(qflat[b][:], 0)
                nc.gpsimd.memset(u2flat[b][:], 0)
                nc.vector.tensor_tensor(
                    out=qflat[b][:].rearrange("p (g j c) -> p g j c",
                                              g=4, c=32)[:, :, :, 0:8],
                    in0=qt_s[:, b * BLK_B:(b + 1) * BLK_B]
                    .rearrange("p (g j) -> p g j", j=4).unsqueeze(3)
                    .broadcast_to([128, 4, 4, 8]),
                    in1=hmask_t.unsqueeze(1).unsqueeze(1)
                    .broadcast_to([128, 4, 4, 8]),
                    op=Alu.mult)

            sc_ps = [None, None]
            lg_ps = [None, None]

            def scores(b):
                ps = psS.tile([128, NK], f32, name="sc_ps")
                sc_ps[b] = ps
                for (lo, hi) in chunks:
                    nc.tensor.matmul(
                        ps[:, lo:hi], sel16b[:],
                        m32b16[:, b * NK + lo:b * NK + hi],
                        start=True, stop=False)
                for jj in range(4):
                    for g in range(4):
                        j = 4 * g + jj
                        kt = kt_t[2 * b + j // GRP]
                        for (lo, hi) in chunks:
                            nc.tensor.matmul(
                                ps[32 * g:32 * g + 32, lo:hi],
                                qflat[b][:, 128 * g + 24 * jj:
                                         128 * g + 24 * jj + 32],
                                kt[:, (j % GRP) * NK + lo:(j % GRP) * NK + hi],
                                start=False, stop=(jj == 3),
                                tile_position=(0, 32 * g))

            def softmax_v_u2(b):
                ps = sc_ps[b]
                negmax = upl_p.tile([128, 1], f32, name="negmax")
                nc.vector.tensor_reduce(out=negmax[:], in_=ps[:], axis=Ax.X,
                                        op=Alu.max, negate=True)
                e2 = e2p.tile([128, NK], f16, name="e2")
                ssum = upl_p.tile([128, 1], f32, name="ssum")
                nc.scalar.activation(e2[:], ps[:], Act.Exp,
                                     bias=negmax[:, 0:1], accum_out=ssum[:])
                rec = upl_p.tile([128, 1], f32, name="rec")
                nc.vector.reciprocal(rec[:], ssum[:])
                usum = upl_p.tile([128, HD], f32, name="usum")

                def vslice(d):
                    vt = vt_t[2 * b + d // (HD // 2)]
                    dd = d % (HD // 2)
                    return vt[:, dd * NK:(dd + 1) * NK]

                # d NDVE..16: gpsimd product (fp16 in, f32 out — the fp16-out
                # path is broken) + ACT Copy+accum.  d 0..NDVE: DVE stt.
                for d in range(NDVE, HD):
                    gjunk = junk_g.tile([128, NK], f32, name="gjunk")
                    nc.gpsimd.tensor_tensor(
                        out=gjunk[:], in0=vslice(d), in1=e2[:], op=Alu.mult)
                    ajunk = junk_a.tile([128, NK], f16, name="ajunk")
                    nc.scalar.activation(ajunk[:], gjunk[:], Act.Copy,
                                         accum_out=usum[:, d:d + 1])
                for d in range(NDVE):
                    sjunk = junk_p.tile([128, NK], f16, name="vjunk")
                    nc.vector.scalar_tensor_tensor(
                        out=sjunk[:], in0=vslice(d), scalar=1.0,
                        in1=e2[:], op0=Alu.mult, op1=Alu.mult,
                        accum_out=usum[:, d:d + 1])
                u_blk = upl_p.tile([128, HD], f32, name="u_blk")
                nc.vector.tensor_tensor(
                    out=u_blk[:], in0=usum[:],
                    in1=rec[:, 0:1].broadcast_to([128, HD]), op=Alu.mult)
                # regroup [(b h), hd] -> [b, (h hd)]: one-hop SBUF->SBUF on
                # the scalar HWDGE ring (ACT queue is near-idle, and HWDGE
                # first-byte latency beats SWDGE by ~0.4us)
                u_plain = upl_p.tile([BLK_B, D], f32, name="u_plain")
                nc.scalar.dma_start(
                    u_plain[:].rearrange("b (h d) -> b h d", h=H), u_blk[:])
                uT_ps = psq.tile([128, 512], f32, name="qp_ps")
                nc.tensor.transpose(uT_ps[:, 0:BLK_B], u_plain[:],
                                    ident_t[0:BLK_B, 0:BLK_B])
                # uT copy + u2 scatter live on ACT (idle here): putting them
                # on DVE queues them ahead of the next block's stt chain,
                # which then stalls ~6us behind the PE-transpose wait.
                uT_sb = upl_p.tile([D, BLK_B], f32, name="uT_sb")
                nc.scalar.copy(uT_sb[:], uT_ps[:, 0:BLK_B])
                u2_ps = psq.tile([128, 512], f32, name="qp_ps")
                nc.tensor.matmul(u2_ps[:, 0:BLK_B], wo_t, uT_sb[:])
                # scatter (u2+bo)/sqrt(D) into the pitch-15 flat stationary
                # (bo_s is pre-scaled so out = u2*rsqd + bo*rsqd)
                nc.scalar.activation(
                    u2flat[b][:].rearrange("p (j c) -> p j c", c=16)
                    [:, :, 0:1],
                    u2_ps[:, 0:BLK_B].unsqueeze(2),
                    Act.Identity, bias=bo_s[:, 0:1], scale=RSQ_D)

            def logits(b):
                ps = psS.tile([BLK_B, NK], f32, name="sc_ps")
                lg_ps[b] = ps
                for (lo, hi) in chunks:
                    nc.tensor.matmul(
                        ps[:, lo:hi], id16b[:],
                        m32b16[:, b * NK + lo:b * NK + hi],
                        start=True, stop=False)
                for j in range(BLK_B):
                    klg = klg_t[4 * b + j // GRPL]
                    for (lo, hi) in chunks:
                        nc.tensor.matmul(
                            ps[:, lo:hi],
                            u2flat[b][:, 15 * j:15 * j + 16],
                            klg[:, (j % GRPL) * NK + lo:(j % GRPL) * NK + hi],
                            start=False, stop=(j == BLK_B - 1))

            def tail(b):
                # pad bias sits inside the psum (tanh(-30) = -1 -> logit -10,
                # strictly below any real 10*tanh), so no mask-add op; the
                # x10 scale folds into the exp bias/scale.
                ps = lg_ps[b]
                tanh_sb = tail_p.tile([BLK_B, NK], f32, name="tanh_sb")
                nc.scalar.activation(tanh_sb[:], ps[:], Act.Tanh)
                max8 = tail_p.tile([BLK_B, 8], f32, name="max8")
                nc.vector.max(max8[:], tanh_sb[:])
                negml = tail_p.tile([BLK_B, 1], f32, name="negml")
                nc.vector.tensor_scalar_mul(negml[:], max8[:, 0:1], -10.0)
                el = tail_p.tile([BLK_B, NK], f32, name="el")
                ssl = tail_p.tile([BLK_B, 1], f32, name="ssl")
                nc.scalar.activation(el[:], tanh_sb[:], Act.Exp,
                                     bias=negml[:, 0:1], scale=10.0,
                                     accum_out=ssl[:])
                probs_sb = tail_p.tile([BLK_B, 1], f32, name="probs_sb")
                nc.vector.reciprocal(probs_sb[:], ssl[:])
                nc.sync.dma_start(probs_out[b * BLK_B:(b + 1) * BLK_B, :],
                                  probs_sb[:])
                selv = tail_p.tile([BLK_B, NK], f32, name="selv")
                nc.vector.scalar_tensor_tensor(
                    out=selv[:], in0=tanh_sb[:], scalar=max8[:, 0:1],
                    in1=idxt_t[:, b * NK:(b + 1) * NK],
                    op0=Alu.is_equal, op1=Alu.mult)
                vert_f = tail_p.tile([BLK_B, 1], f32, name="vert_f")
                nc.vector.tensor_reduce(out=vert_f[:], in_=selv[:], axis=Ax.X,
                                        op=Alu.max)
                nc.sync.dma_start(vert_out[b * BLK_B:(b + 1) * BLK_B, :],
                                  vert_f[:])

            scores(0)
            softmax_v_u2(0)
            scores(1)
            softmax_v_u2(1)
            logits(0)
            tail(0)
            logits(1)
            tail(1)

    nc.finalize()
    return nc


def _get_program(NK):
    if NK not in _PROG_CACHE:
        _PROG_CACHE[NK] = _build_program(NK)
    return _PROG_CACHE[NK]


def _fill_shared(inputs):
    f32 = np.float32
    Wq = np.asarray(inputs["Wq"], f32)
    bq = np.asarray(inputs["bq"], f32)
    Wo = np.asarray(inputs["Wo"], f32)
    bo = np.asarray(inputs["bo"], f32)
    wqT = np.zeros((KPAD, D), f32)
    wqT[: 3 * D + 2] = Wq.T
    hmask = np.zeros((128, H), f32)
    for h in range(H):
        hmask[h * HD:(h + 1) * HD, h] = 1.0
    sel16 = np.zeros((BLK_B, 128), f32)
    for b in range(BLK_B):
        sel16[b, b * H:(b + 1) * H] = 1.0
    _SHARED_CACHE.update({
        "wqT": wqT,
        "bq": bq.reshape(D, 1),
        "woT": np.ascontiguousarray(Wo.T),
        "bo": bo.reshape(D, 1),
        "ident": np.eye(128, dtype=f32),
        "hmask": hmask,
        "sel16": sel16,
    })


def _prep_core_inputs(inputs, core, NK):
    """Pure layout transforms + mask compaction for one core's batch slice."""
    f32 = np.float32
    f16 = np.float16
    sl = slice(core * BPC, (core + 1) * BPC)
    h_g = np.asarray(inputs["h_g"], f32)[sl]
    first = np.asarray(inputs["first"], f32)[sl]
    last = np.asarray(inputs["last"], f32)[sl]
    context = np.asarray(inputs["context"], f32)[sl]
    K = np.asarray(inputs["K"], f32)[sl]
    V = np.asarray(inputs["V"], f32)[sl]
    K_lg = np.asarray(inputs["K_lg"], f32)[sl]
    mask = np.asarray(inputs["mask"], np.int32)[sl]

    h_c = np.concatenate([h_g, first, last, context], axis=1)      # [32, 386]
    hcT = np.zeros((KPAD, BPC), f32)
    hcT[: 3 * D + 2] = h_c.T

    sh = _SHARED_CACHE
    consts = np.zeros((128, NCONST), f32)
    consts[:, 0:128] = sh["ident"]
    consts[:, 128:256] = sh["woT"]
    consts[:, 256:257] = sh["bq"]
    consts[:, 257:258] = sh["bo"]
    consts[:, 258:386] = hcT.reshape(4, 128, BPC).transpose(1, 0, 2) \
        .reshape(128, 4 * BPC)
    consts[:, 386:898] = sh["wqT"].reshape(4, 128, D).transpose(1, 0, 2) \
        .reshape(128, 4 * D)
    consts[:, 898:906] = sh["hmask"]
    consts[0:BLK_B, 906:1034] = sh["sel16"]

    # --- mask compaction: keep only unmasked columns, pad to NK ---
    G = np.zeros((BPC, NK), np.int64)          # gather indices (pad -> 0)
    pad = np.full((BPC, NK), f32(NEG), f32)    # 0 kept / -1e15 pad bias
    idxt = np.zeros((BPC, NK), f32)            # original position ids
    for b in range(BPC):
        idx = np.nonzero(mask[b])[0]
        n = len(idx)
        G[b, :n] = idx
        pad[b, :n] = 0.0
        idxt[b, :n] = idx.astype(f32)
    keep = (pad == 0.0)

    Kc = np.take_along_axis(K, G[:, None, :, None], axis=2)   # [32,8,NK,16]
    Vc = np.take_along_axis(V, G[:, None, :, None], axis=2)
    Lc = np.take_along_axis(K_lg, G[:, :, None], axis=1)      # [32,NK,128]
    # zero the pad columns so their matmul/attn contributions are exact 0
    Kc *= keep[:, None, :, None]
    Vc *= keep[:, None, :, None]
    Lc *= keep[:, :, None]

    Kt = Kc.transpose(0, 1, 3, 2).reshape(BPC, D, NK)         # [b,(h d),n]
    KtG = np.ascontiguousarray(
        Kt.reshape(4, GRP, 128, NK).transpose(0, 2, 1, 3)
        .reshape(4, 128, GRP * NK).astype(f16))
    Vt = np.ascontiguousarray(
        Vc.transpose(0, 1, 3, 2).reshape(2, 128, 2, (HD // 2) * NK)
        .transpose(0, 2, 1, 3).reshape(4, 128, (HD // 2) * NK).astype(f16))
    Lt = Lc.transpose(0, 2, 1)                                # [b, d, n]
    KlgG = np.ascontiguousarray(
        Lt.reshape(8, GRPL, 128, NK).transpose(0, 2, 1, 3)
        .reshape(8, 128, GRPL * NK).astype(f16))

    def block_major(a):                        # [32, NK] -> [16, 2*NK]
        return np.ascontiguousarray(
            a.reshape(2, BLK_B, NK).transpose(1, 0, 2).reshape(BLK_B, 2 * NK))

    return {
        "consts": consts,
        "KtG": KtG,
        "Vt": Vt,
        "KlgG": KlgG,
        "m32b": block_major(pad),
        "idxt": block_major(idxt),
    }


def make_in_maps(inputs, NK):
    _fill_shared(inputs)
    return [_prep_core_inputs(inputs, c, NK) for c in range(NCORES)]


def _pick_nk(inputs):
    mask = np.asarray(inputs["mask"])
    mx = int((mask != 0).sum(axis=1).max())
    return max(128, -(-mx // 64) * 64)


def _assemble(results):
    verts = np.concatenate(
        [np.rint(np.asarray(r["verts"], np.float32)).astype(np.int32)
         for r in results])
    probs = np.concatenate([np.asarray(r["probs"], np.float32) for r in results])
    return verts.reshape(B, 1), probs.reshape(B, 1)


def run_spmd(inputs, trace=False, **kw):
    from concourse.bass_utils import run_bass_kernel_spmd

    NK = _pick_nk(inputs)
    nc = _get_program(NK)
    in_maps = make_in_maps(inputs, NK)
    br = run_bass_kernel_spmd(nc, in_maps, list(range(NCORES)), trace=trace, **kw)
    return br


def kernel(**inputs):
    br = run_spmd(inputs, trace=False)
    return _assemble(br.results)


# revision 37
# speedup vs baseline: 1.0546x; 1.0546x over previous
"""Trainium2 Bass kernel for nn_Decoder (single-query MHA + pointer head).

Contract: kernel(**inputs) takes the FULL unsharded numpy inputs (as produced
by the problem's setup_inputs) and returns the full output (vertexes, probs),
matching the reference up to fp32 rounding.

v9 strategy (pure data parallelism over batch, 8 NeuronCores, 32 batch each):
  - Host-side compaction: mask kills ~50% of the N=1024 positions (score
    -1e15 -> attn weight 0; pointer logit -1e15 -> never argmax), so only
    the unmasked K/V/K_lg columns are shipped, padded to N_k (multiple of
    64, 576 for the seed-0 inputs).  Pad columns are zero + a -1e15 bias.
    Original vertex ids are recovered on-device from an index table via
    (logit == rowmax) * idx -> reduce-max.
  - All scores and pointer logits run on the TensorEngine as PSUM
    accumulation chains with zero-padded per-batch stationaries.  Scores
    use [128, 32] stationaries on tile_position col-bands (pitch-24 flat
    windows put batch 4g+jj's q at band 32g, local col 8jj+h -> psum
    partition 8j+h); emission is jj-outer so the four bands run
    concurrently in the PE array.  The pad bias is accumulated by one
    bf16 selector matmul per bank, so each block's scores drain with one
    DVE reduce (negmax) + one ACT exp.  A burst of junk bf16 matmuls
    warms the PE's HAM clock gate before the first score chain.
  - V contraction: 12 d's as DVE stt+accum, 4 d's as DVE tensor_tensor
    product + ACT Copy+accum_out, balancing the two engines.
  - Pointer head runs per 16-batch block ([16, N_k] PSUM reusing the
    score banks), so block 0's tanh/softmax/argmax tail hides under the
    DMA stream and only block 1's short tail trails the last K_lg tile
    (a 4-batch group).  exp bias comes from negating max8's first lane.
  - One HWDGE DMA ring (sync) carries all bulk loads in need order
    (~420 GB/s observed); V ships as four quarter tiles so the first stt
    can start earlier; u round-trips are one-hop SBUF->SBUF DMAs on the
    scalar HWDGE ring.
"""

import numpy as np

B, N, D, H, HD = 256, 1024, 128, 8, 16
NCORES = 8
BPC = B // NCORES          # 32 batches per core
BLK_B = 16                 # batches per score-block (16 b x 8 h = 128 rows)
GRP = 8                    # batches per K^T DMA group tile
GRPL = 4                   # batches per K_lg DMA group tile
KPAD = 512                 # 386 -> 512 (4 chunks of 128) for Q projection
NEG = -30.0                # pad bias: exp(-30+s-max) underflows fp16 to 0
                           # exactly in the attn softmax, and tanh(-30) = -1
                           # exactly in f32, so pads sit at pointer logit -10
                           # (strictly below every real 10*tanh) and add only
                           # ~exp(-10-max) to the pointer softmax sum.
RSQ_D = float(1.0 / np.sqrt(128.0))
NCONST = 1034              # ident|woT|bq|bo|hcT|wqT|hmask|sel16
NDVE = 16                  # V-contraction d's on DVE stt (gpsimd tensor ops
                           # concurrent with DVE wedge the device; ACT-assist
                           # is slower than stt: ~1040ns vs 755ns per d)

_PROG_CACHE = {}
_SHARED_CACHE = {}


def _build_program(NK):
    import concourse.bass as bass
    import concourse.bacc as bacc
    import concourse.mybir as mybir
    from concourse.tile import TileContext

    f32 = mybir.dt.float32
    f16 = mybir.dt.float16
    bf16 = mybir.dt.bfloat16
    Alu = mybir.AluOpType
    Act = mybir.ActivationFunctionType
    Ax = mybir.AxisListType

    # psum bank chunks of the N_k columns
    chunks = [(0, min(512, NK))]
    if NK > 512:
        chunks.append((512, NK))

    nc = bacc.Bacc(None, target_bir_lowering=False)

    consts = nc.declare_dram_parameter("consts", [128, NCONST], f32,
                                       isOutput=False)
    KtG = nc.declare_dram_parameter("KtG", [4, 128, GRP * NK], f16,
                                    isOutput=False)
    Vt = nc.declare_dram_parameter("Vt", [4, 128, (HD // 2) * NK], f16,
                                   isOutput=False)
    KlgG = nc.declare_dram_parameter("KlgG", [8, 128, GRPL * NK], f16,
                                     isOutput=False)
    m32b = nc.declare_dram_parameter("m32b", [BLK_B, 2 * NK], f32,
                                     isOutput=False)
    idxt = nc.declare_dram_parameter("idxt", [BLK_B, 2 * NK], f32,
                                     isOutput=False)
    vert_out = nc.declare_dram_parameter("verts", [BPC, 1], f32, isOutput=True)
    probs_out = nc.declare_dram_parameter("probs", [BPC, 1], f32, isOutput=True)

    with TileContext(nc) as tc:
        import contextlib

        with contextlib.ExitStack() as ctx:
            const_p = ctx.enter_context(tc.tile_pool(name="const", bufs=1))
            small_p = ctx.enter_context(tc.tile_pool(name="small", bufs=1))
            ktp = ctx.enter_context(tc.tile_pool(name="ktp", bufs=4))
            vtp = ctx.enter_context(tc.tile_pool(name="vtp", bufs=4))
            klgp = ctx.enter_context(tc.tile_pool(name="klgp", bufs=8))
            e2p = ctx.enter_context(tc.tile_pool(name="e2p", bufs=2))
            junk_p = ctx.enter_context(tc.tile_pool(name="junk", bufs=3))
            junk_a = ctx.enter_context(tc.tile_pool(name="junka", bufs=3))
            junk_g = ctx.enter_context(tc.tile_pool(name="junkg", bufs=3))
            upl_p = ctx.enter_context(tc.tile_pool(name="upl", bufs=2))
            tail_p = ctx.enter_context(tc.tile_pool(name="tail", bufs=2))
            psq = ctx.enter_context(
                tc.tile_pool(name="psq", bufs=2, space=bass.MemorySpace.PSUM))
            psS = ctx.enter_context(
                tc.tile_pool(name="psS", bufs=2, space=bass.MemorySpace.PSUM))

            # ====== DMA: one sync-ring stream in need order ======
            cblob = const_p.tile([128, NCONST], f32, name="cblob")
            nc.sync.dma_start(cblob[:], consts[:])
            m32b_t = small_p.tile([BLK_B, 2 * NK], f32)
            nc.sync.dma_start(m32b_t[:], m32b[:])
            kt_t = [ktp.tile([128, GRP * NK], f16, name="kt_t")
                    for _ in range(4)]
            vt_t = [vtp.tile([128, (HD // 2) * NK], f16, name="vt_t")
                    for _ in range(4)]
            klg_t = [klgp.tile([128, GRPL * NK], f16, name="klg_t")
                     for _ in range(8)]
            nc.sync.dma_start(kt_t[0][:], KtG[0])
            nc.sync.dma_start(kt_t[1][:], KtG[1])
            nc.sync.dma_start(vt_t[0][:], Vt[0])
            nc.sync.dma_start(vt_t[1][:], Vt[1])
            nc.sync.dma_start(kt_t[2][:], KtG[2])
            nc.sync.dma_start(kt_t[3][:], KtG[3])
            nc.sync.dma_start(klg_t[0][:], KlgG[0])
            nc.sync.dma_start(klg_t[1][:], KlgG[1])
            nc.sync.dma_start(vt_t[2][:], Vt[2])
            nc.sync.dma_start(vt_t[3][:], Vt[3])
            for g in range(2, 8):
                nc.sync.dma_start(klg_t[g][:], KlgG[g])
            idxt_t = small_p.tile([BLK_B, 2 * NK], f32)
            nc.sync.dma_start(idxt_t[:], idxt[:])

            ident_t = cblob[:, 0:128]
            wo_t = cblob[:, 128:256]
            bq_t = cblob[:, 256:257]
            bo_t = cblob[:, 257:258]
            hc_t = cblob[:, 258:386].rearrange("p (c b) -> p c b", b=BPC)
            wq_t = cblob[:, 386:898].rearrange("p (c d) -> p c d", d=D)
            hmask_t = cblob[:, 898:906]

            # preload the ACT function tables off the critical path
            dummy = small_p.tile([1, 16], f32)
            nc.vector.memset(dummy[:], 0)
            nc.scalar.activation(dummy[:], dummy[:], Act.Exp)
            nc.scalar.activation(dummy[:], dummy[:], Act.Tanh)

            # bf16 casts for the pad-bias matmul operands
            sel16b = const_p.tile([BLK_B, 128], bf16)
            nc.vector.tensor_copy(sel16b[:], cblob[0:BLK_B, 906:1034])
            id16b = const_p.tile([BLK_B, BLK_B], bf16)
            nc.vector.tensor_copy(id16b[:], cblob[0:BLK_B, 0:BLK_B])
            m32b16 = small_p.tile([BLK_B, 2 * NK], bf16)
            nc.vector.tensor_copy(m32b16[:], m32b_t[:])

            # ====== Q projection -> qt_s = 0.25*(Q^T + bq)  [(h d), b] ======
            qp_ps = psq.tile([128, 512], f32, name="qp_ps")
            for kc in range(KPAD // 128):
                nc.tensor.matmul(
                    qp_ps[:, 0:BPC], wq_t[:, kc, :], hc_t[:, kc, :],
                    start=(kc == 0), stop=(kc == KPAD // 128 - 1))
            bq25 = const_p.tile([D, 1], f32)
            nc.vector.tensor_scalar_mul(bq25[:], bq_t, 0.25)
            bo_s = const_p.tile([D, 1], f32)
            nc.vector.tensor_scalar_mul(bo_s[:], bo_t, RSQ_D)
            qt_s = small_p.tile([D, BPC], f32)
            nc.vector.scalar_tensor_tensor(
                out=qt_s[:], in0=qp_ps[:, 0:BPC], scalar=0.25,
                in1=bq25[:, 0:1].broadcast_to([D, BPC]),
                op0=Alu.mult, op1=Alu.add)

            # ====== zero-padded stationaries ======
            # scores: block-local batch j = 4g+jj -> band g (tile_position
            # (0,32g)), window qflat[:, 128g+24jj : +32], nonzero flat col
            # 128g+32jj+h = local col 8jj+h -> psum partition 8j+h.
            qflat = [small_p.tile([128, 512], f16, name=f"qflat{b}")
                     for b in range(2)]
            # logits: batch j at flat col 16j inside a pitch-15 window
            # [15j, 15j+16) -> local col j -> psum partition j.
            u2flat = [small_p.tile([128, 256], f16, name=f"u2flat{b}")
                      for b in range(2)]
            for b in range(2):
                nc.gpsimd.memset(qflat[b][:], 0)
                nc.gpsimd.memset(u2flat[b][:], 0)
                nc.vector.tensor_tensor(
                    out=qflat[b][:].rearrange("p (g j c) -> p g j c",
                                              g=4, c=32)[:, :, :, 0:8],
                    in0=qt_s[:, b * BLK_B:(b + 1) * BLK_B]
                    .rearrange("p (g j) -> p g j", j=4).unsqueeze(3)
                    .broadcast_to([128, 4, 4, 8]),
                    in1=hmask_t.unsqueeze(1).unsqueeze(1)
                    .broadcast_to([128, 4, 4, 8]),
                    op=Alu.mult)

            sc_ps = [None, None]
            lg_ps = [None, None]

            def scores(b):
                ps = psS.tile([128, NK], f32, name="sc_ps")
                sc_ps[b] = ps
                for (lo, hi) in chunks:
                    nc.tensor.matmul(
                        ps[:, lo:hi], sel16b[:],
                        m32b16[:, b * NK + lo:b * NK + hi],
                        start=True, stop=False)
                for jj in range(4):
                    for g in range(4):
                        j = 4 * g + jj
                        kt = kt_t[2 * b + j // GRP]
                        for (lo, hi) in chunks:
                            nc.tensor.matmul(
                                ps[32 * g:32 * g + 32, lo:hi],
                                qflat[b][:, 128 * g + 24 * jj:
                                         128 * g + 24 * jj + 32],
                                kt[:, (j % GRP) * NK + lo:(j % GRP) * NK + hi],
                                start=False, stop=(jj == 3),
                                tile_position=(0, 32 * g))

            u_blks = [None, None]

            def softmax_v(b):
                ps = sc_ps[b]
                negmax = upl_p.tile([128, 1], f32, name="negmax")
                nc.vector.tensor_reduce(out=negmax[:], in_=ps[:], axis=Ax.X,
                                        op=Alu.max, negate=True)
                e2 = e2p.tile([128, NK], f16, name="e2")
                ssum = upl_p.tile([128, 1], f32, name="ssum")
                nc.scalar.activation(e2[:], ps[:], Act.Exp,
                                     bias=negmax[:, 0:1], accum_out=ssum[:])
                rec = upl_p.tile([128, 1], f32, name="rec")
                nc.vector.reciprocal(rec[:], ssum[:])
                usum = upl_p.tile([128, HD], f32, name="usum")

                def vslice(d):
                    vt = vt_t[2 * b + d // (HD // 2)]
                    dd = d % (HD // 2)
                    return vt[:, dd * NK:(dd + 1) * NK]

                # d NDVE..16: gpsimd product (fp16 in, f32 out — the fp16-out
                # path is broken) + ACT Copy+accum.  d 0..NDVE: DVE stt.
                for d in range(NDVE, HD):
                    gjunk = junk_g.tile([128, NK], f32, name="gjunk")
                    nc.gpsimd.tensor_tensor(
                        out=gjunk[:], in0=vslice(d), in1=e2[:], op=Alu.mult)
                    ajunk = junk_a.tile([128, NK], f16, name="ajunk")
                    nc.scalar.activation(ajunk[:], gjunk[:], Act.Copy,
                                         accum_out=usum[:, d:d + 1])
                for d in range(NDVE):
                    sjunk = junk_p.tile([128, NK], f16, name="vjunk")
                    nc.vector.scalar_tensor_tensor(
                        out=sjunk[:], in0=vslice(d), scalar=1.0,
                        in1=e2[:], op0=Alu.mult, op1=Alu.mult,
                        accum_out=usum[:, d:d + 1])
                u_blk = upl_p.tile([128, HD], f32, name="u_blk")
                nc.vector.tensor_tensor(
                    out=u_blk[:], in0=usum[:],
                    in1=rec[:, 0:1].broadcast_to([128, HD]), op=Alu.mult)
                u_blks[b] = u_blk

            def u2path(b):
                # Emitted AFTER both blocks' V-phases: the tiny uT-copy and
                # u2-scatter DVE ops then sit BEHIND the stt chains in the
                # DVE FIFO, so the chains can no longer stall behind the
                # PE-transpose wait (the 6us DVE gap in the v13 trace).
                u_blk = u_blks[b]
                # regroup [(b h), hd] -> [b, (h hd)]: one-hop SBUF->SBUF on
                # the otherwise-idle SWDGE ring (keeps the ACT queue clear)
                u_plain = upl_p.tile([BLK_B, D], f32, name="u_plain")
                nc.gpsimd.dma_start(
                    u_plain[:].rearrange("b (h d) -> b h d", h=H), u_blk[:])
                uT_ps = psq.tile([128, 512], f32, name="qp_ps")
                nc.tensor.transpose(uT_ps[:, 0:BLK_B], u_plain[:],
                                    ident_t[0:BLK_B, 0:BLK_B])
                uT_sb = upl_p.tile([D, BLK_B], f32, name="uT_sb")
                nc.vector.tensor_copy(uT_sb[:], uT_ps[:, 0:BLK_B])
                u2_ps = psq.tile([128, 512], f32, name="qp_ps")
                nc.tensor.matmul(u2_ps[:, 0:BLK_B], wo_t, uT_sb[:])
                # scatter (u2+bo)/sqrt(D) into the pitch-15 flat stationary
                # (DVE stt; bo_s is pre-scaled so out = u2*rsqd + bo*rsqd)
                nc.vector.scalar_tensor_tensor(
                    out=u2flat[b][:].rearrange("p (j c) -> p j c", c=16)
                    [:, :, 0:1],
                    in0=u2_ps[:, 0:BLK_B].unsqueeze(2), scalar=RSQ_D,
                    in1=bo_s[:, 0:1].unsqueeze(2)
                    .broadcast_to([128, BLK_B, 1]),
                    op0=Alu.mult, op1=Alu.add)

            def logits(b):
                ps = psS.tile([BLK_B, NK], f32, name="sc_ps")
                lg_ps[b] = ps
                for (lo, hi) in chunks:
                    nc.tensor.matmul(
                        ps[:, lo:hi], id16b[:],
                        m32b16[:, b * NK + lo:b * NK + hi],
                        start=True, stop=False)
                for j in range(BLK_B):
                    klg = klg_t[4 * b + j // GRPL]
                    for (lo, hi) in chunks:
                        nc.tensor.matmul(
                            ps[:, lo:hi],
                            u2flat[b][:, 15 * j:15 * j + 16],
                            klg[:, (j % GRPL) * NK + lo:(j % GRPL) * NK + hi],
                            start=False, stop=(j == BLK_B - 1))

            def tail(b):
                # pad bias sits inside the psum (tanh(-30) = -1 -> logit -10,
                # strictly below any real 10*tanh), so no mask-add op; the
                # x10 scale folds into the exp bias/scale.
                ps = lg_ps[b]
                tanh_sb = tail_p.tile([BLK_B, NK], f32, name="tanh_sb")
                nc.scalar.activation(tanh_sb[:], ps[:], Act.Tanh)
                max8 = tail_p.tile([BLK_B, 8], f32, name="max8")
                nc.vector.max(max8[:], tanh_sb[:])
                negml = tail_p.tile([BLK_B, 1], f32, name="negml")
                nc.vector.tensor_scalar_mul(negml[:], max8[:, 0:1], -10.0)
                el = tail_p.tile([BLK_B, NK], f32, name="el")
                ssl = tail_p.tile([BLK_B, 1], f32, name="ssl")
                nc.scalar.activation(el[:], tanh_sb[:], Act.Exp,
                                     bias=negml[:, 0:1], scale=10.0,
                                     accum_out=ssl[:])
                probs_sb = tail_p.tile([BLK_B, 1], f32, name="probs_sb")
                nc.vector.reciprocal(probs_sb[:], ssl[:])
                nc.sync.dma_start(probs_out[b * BLK_B:(b + 1) * BLK_B, :],
                                  probs_sb[:])
                selv = tail_p.tile([BLK_B, NK], f32, name="selv")
                nc.vector.scalar_tensor_tensor(
                    out=selv[:], in0=tanh_sb[:], scalar=max8[:, 0:1],
                    in1=idxt_t[:, b * NK:(b + 1) * NK],
                    op0=Alu.is_equal, op1=Alu.mult)
                vert_f = tail_p.tile([BLK_B, 1], f32, name="vert_f")
                nc.vector.tensor_reduce(out=vert_f[:], in_=selv[:], axis=Ax.X,
                                        op=Alu.max)
                nc.sync.dma_start(vert_out[b * BLK_B:(b + 1) * BLK_B, :],
                                  vert_f[:])

            scores(0)
            softmax_v(0)
            scores(1)
            softmax_v(1)
            u2path(0)
            u2path(1)
            logits(0)
            tail(0)
            logits(1)
            tail(1)

    nc.finalize()
    return nc


def _get_program(NK):
    if NK not in _PROG_CACHE:
        _PROG_CACHE[NK] = _build_program(NK)
    return _PROG_CACHE[NK]


def _fill_shared(inputs):
    f32 = np.float32
    Wq = np.asarray(inputs["Wq"], f32)
    bq = np.asarray(inputs["bq"], f32)
    Wo = np.asarray(inputs["Wo"], f32)
    bo = np.asarray(inputs["bo"], f32)
    wqT = np.zeros((KPAD, D), f32)
    wqT[: 3 * D + 2] = Wq.T
    hmask = np.zeros((128, H), f32)
    for h in range(H):
        hmask[h * HD:(h + 1) * HD, h] = 1.0
    sel16 = np.zeros((BLK_B, 128), f32)
    for b in range(BLK_B):
        sel16[b, b * H:(b + 1) * H] = 1.0
    _SHARED_CACHE.update({
        "wqT": wqT,
        "bq": bq.reshape(D, 1),
        "woT": np.ascontiguousarray(Wo.T),
        "bo": bo.reshape(D, 1),
        "ident": np.eye(128, dtype=f32),
        "hmask": hmask,
        "sel16": sel16,
    })


def _prep_core_inputs(inputs, core, NK):
    """Pure layout transforms + mask compaction for one core's batch slice."""
    f32 = np.float32
    f16 = np.float16
    sl = slice(core * BPC, (core + 1) * BPC)
    h_g = np.asarray(inputs["h_g"], f32)[sl]
    first = np.asarray(inputs["first"], f32)[sl]
    last = np.asarray(inputs["last"], f32)[sl]
    context = np.asarray(inputs["context"], f32)[sl]
    K = np.asarray(inputs["K"], f32)[sl]
    V = np.asarray(inputs["V"], f32)[sl]
    K_lg = np.asarray(inputs["K_lg"], f32)[sl]
    mask = np.asarray(inputs["mask"], np.int32)[sl]

    h_c = np.concatenate([h_g, first, last, context], axis=1)      # [32, 386]
    hcT = np.zeros((KPAD, BPC), f32)
    hcT[: 3 * D + 2] = h_c.T

    sh = _SHARED_CACHE
    consts = np.zeros((128, NCONST), f32)
    consts[:, 0:128] = sh["ident"]
    consts[:, 128:256] = sh["woT"]
    consts[:, 256:257] = sh["bq"]
    consts[:, 257:258] = sh["bo"]
    consts[:, 258:386] = hcT.reshape(4, 128, BPC).transpose(1, 0, 2) \
        .reshape(128, 4 * BPC)
    consts[:, 386:898] = sh["wqT"].reshape(4, 128, D).transpose(1, 0, 2) \
        .reshape(128, 4 * D)
    consts[:, 898:906] = sh["hmask"]
    consts[0:BLK_B, 906:1034] = sh["sel16"]

    # --- mask compaction: keep only unmasked columns, pad to NK ---
    G = np.zeros((BPC, NK), np.int64)          # gather indices (pad -> 0)
    pad = np.full((BPC, NK), f32(NEG), f32)    # 0 kept / -1e15 pad bias
    idxt = np.zeros((BPC, NK), f32)            # original position ids
    for b in range(BPC):
        idx = np.nonzero(mask[b])[0]
        n = len(idx)
        G[b, :n] = idx
        pad[b, :n] = 0.0
        idxt[b, :n] = idx.astype(f32)
    keep = (pad == 0.0)

    Kc = np.take_along_axis(K, G[:, None, :, None], axis=2)   # [32,8,NK,16]
    Vc = np.take_along_axis(V, G[:, None, :, None], axis=2)
    Lc = np.take_along_axis(K_lg, G[:, :, None], axis=1)      # [32,NK,128]
    # zero the pad columns so their matmul/attn contributions are exact 0
    Kc *= keep[:, None, :, None]
    Vc *= keep[:, None, :, None]
    Lc *= keep[:, :, None]

    Kt = Kc.transpose(0, 1, 3, 2).reshape(BPC, D, NK)         # [b,(h d),n]
    KtG = np.ascontiguousarray(
        Kt.reshape(4, GRP, 128, NK).transpose(0, 2, 1, 3)
        .reshape(4, 128, GRP * NK).astype(f16))
    Vt = np.ascontiguousarray(
        Vc.transpose(0, 1, 3, 2).reshape(2, 128, 2, (HD // 2) * NK)
        .transpose(0, 2, 1, 3).reshape(4, 128, (HD // 2) * NK).astype(f16))
    Lt = Lc.transpose(0, 2, 1)                                # [b, d, n]
    KlgG = np.ascontiguousarray(
        Lt.reshape(8, GRPL, 128, NK).transpose(0, 2, 1, 3)
        .reshape(8, 128, GRPL * NK).astype(f16))

    def block_major(a):                        # [32, NK] -> [16, 2*NK]
        return np.ascontiguousarray(
            a.reshape(2, BLK_B, NK).transpose(1, 0, 2).reshape(BLK_B, 2 * NK))

    return {
        "consts": consts,
        "KtG": KtG,
        "Vt": Vt,
        "KlgG": KlgG,
        "m32b": block_major(pad),
        "idxt": block_major(idxt),
    }


def make_in_maps(inputs, NK):
    _fill_shared(inputs)
    return [_prep_core_inputs(inputs, c, NK) for c in range(NCORES)]


def _pick_nk(inputs):
    mask = np.asarray(inputs["mask"])
    mx = int((mask != 0).sum(axis=1).max())
    return max(128, -(-mx // 64) * 64)


def _assemble(results):
    verts = np.concatenate(
        [np.rint(np.asarray(r["verts"], np.float32)).astype(np.int32)
         for r in results])
    probs = np.concatenate([np.asarray(r["probs"], np.float32) for r in results])
    return verts.reshape(B, 1), probs.reshape(B, 1)


def run_spmd(inputs, trace=False, **kw):
    from concourse.bass_utils import run_bass_kernel_spmd

    NK = _pick_nk(inputs)
    nc = _get_program(NK)
    in_maps = make_in_maps(inputs, NK)
    br = run_bass_kernel_spmd(nc, in_maps, list(range(NCORES)), trace=trace, **kw)
    return br


def kernel(**inputs):
    br = run_spmd(inputs, trace=False)
    return _assemble(br.results)
